# revision 18
# baseline (speedup 1.0000x reference)
"""BlockEqLinear kernel for Trainium2 (8 NeuronCores, SPMD data-parallel over batch).

Math (reference):
    x: [4096, 4096] viewed as [B=4096, K=8, H=512]
    A, B: [G=4, H, H]
    out[b, g, k, :] = x_k[b] @ (A_g - B_g)^T + S[b] @ B_g^T,  S = sum_k x_k
    returned as [B, G*K*H] = [4096, 16384]

Strategy:
  - Shard batch across 8 cores (512 rows each); weights replicated.
  - Host-side layout prep: partition-major transposes so the contraction
    dim (h) is the partition dim on chip; precompute D = A - B.
  - Phase A (per core): tsum[bt,g] = S^T.T @ B_g^T  (16 groups of 4
    accumulating fp32r matmuls), evicted to SBUF.
  - Phase B: k-outer so x^T streams in just-in-time one k-slice at a
    time:  for k: for bt: for g: 4 matmuls -> psum; DVE adds tsum and
    packs 4 g-slices into one staging tile; one DMA per (k, bt).
  - fp32r (FP22) matmuls: full PE rate at moving-dim 512.
  - Output written as bf16 on device (halves write traffic; PE-bound
    after that), upcast to fp32 on host.
"""

import numpy as np

import concourse.bass as bass
import concourse.mybir as mybir
import concourse.tile as tile
from concourse import bacc
from concourse.bass_utils import run_bass_kernel_spmd
from contextlib import ExitStack

G, K, H = 4, 8, 512
B_TOTAL = 4096
NCORES = 8
BS = B_TOTAL // NCORES  # 512 batch rows per core
P = 128                 # partition dim
HC = H // P             # 4 contraction chunks per 512-dim h
NBT = BS // P           # 4 b-tiles per core

F32 = mybir.dt.float32
F32R = mybir.dt.float32r
BF16 = mybir.dt.bfloat16

OUT_BF16 = True         # write y as bf16 on device, upcast on host
MM_BF16 = True          # all matmul operands (x^T, S^T, weights) in bf16:
                        # halves input DMA, enables fast weight load
N_WARMUP = 5            # dummy PE matmuls to warm HAM during input DMA

_CACHE = {}


def _build():
    out_dt = BF16 if OUT_BF16 else F32

    nc = bacc.Bacc(
        "TRN2", target_bir_lowering=False, debug=False, num_devices=NCORES
    )

    # Host-packed, partition-major inputs:
    #   xt[q, k, hc, b]   = x[b, k*512 + hc*128 + q]
    #   st[q, hc, b]      = S[b, hc*128 + q]
    #   dtw[q, g, hc, p]  = (A - B)[g, p, hc*128 + q]
    #   btw[q, g, hc, p]  = B[g, p, hc*128 + q]
    x_dt = BF16 if MM_BF16 else F32R
    xt = nc.dram_tensor("xt", [P, K, HC, BS], x_dt, kind="ExternalInput")
    st = nc.dram_tensor("st", [P, HC, BS], x_dt, kind="ExternalInput")
    dtw = nc.dram_tensor("dtw", [P, G, HC, H], x_dt, kind="ExternalInput")
    btw = nc.dram_tensor("btw", [P, G, HC, H], x_dt, kind="ExternalInput")
    # y_dev[bt, k, p, g*512 + pp] = out[bt*128 + p, g, k, pp]
    y = nc.dram_tensor("y", [NBT, K, P, G * H], out_dt, kind="ExternalOutput")

    with tile.TileContext(nc) as tc, ExitStack() as ctx:
        wpool = ctx.enter_context(tc.tile_pool(name="w", bufs=1))
        xpool = ctx.enter_context(tc.tile_pool(name="x", bufs=1))
        tsump = ctx.enter_context(tc.tile_pool(name="tsum", bufs=1))
        opool = ctx.enter_context(tc.tile_pool(name="o", bufs=8))
        psd = ctx.enter_context(tc.tile_pool(name="psd", bufs=6, space="PSUM"))
        pss = ctx.enter_context(tc.tile_pool(name="pss", bufs=2, space="PSUM"))

        # PE warm-up scratch: zeroed tile for dummy matmuls (below) that
        # run while the first input DMAs are in flight, so HAM is at
        # K=8/8 (2.4 GHz) when the real matmul stream starts.
        scratch = wpool.tile([P, H], BF16)
        nc.gpsimd.memset(scratch[:], 0.0)

        # SBUF-resident tensors. Two HWDGE queues in parallel:
        # sync carries phase-A deps (st, btw), scalar carries phase-B
        # deps (dtw, xt) — so early arrivals overlap.
        st_sb = xpool.tile([P, HC * BS], x_dt)
        nc.sync.dma_start(st_sb[:], st[:, :, :])
        bt_sb = wpool.tile([P, G * HC * H], x_dt)
        for g in range(G):
            nc.sync.dma_start(
                bt_sb[:, g * HC * H : (g + 1) * HC * H], btw[:, g, :, :]
            )
        dt_sb = wpool.tile([P, G * HC * H], x_dt)
        for g in range(G):
            nc.scalar.dma_start(
                dt_sb[:, g * HC * H : (g + 1) * HC * H], dtw[:, g, :, :]
            )
        xt_sb = xpool.tile([P, K * HC * BS], x_dt)
        for k in range(K):
            nc.scalar.dma_start(
                xt_sb[:, k * HC * BS : (k + 1) * HC * BS], xt[:, k, :, :]
            )

        # Dummy warm-up matmuls (PE program order puts these before the
        # real stream; they execute during the input-DMA wait).
        warm_ps = pss.tile([P, H], F32, tag="ps")
        for i in range(N_WARMUP):
            nc.tensor.matmul(
                warm_ps[:],
                scratch[:, :P],
                scratch[:],
                start=True,
                stop=True,
            )

        # Phase A: tsum[bt, g] = S-tile @ B_g^T for all 16 (bt, g) pairs.
        # g-major so the first 16 matmuls need only st + btw[g=0].
        tsum_sb = tsump.tile([P, NBT * G * H], BF16)
        for g in range(G):
            for bt in range(NBT):
                b0 = bt * P
                ps = pss.tile([P, H], F32)
                for hc in range(HC):
                    nc.tensor.matmul(
                        ps[:],
                        st_sb[:, hc * BS + b0 : hc * BS + b0 + P],
                        bt_sb[:, (g * HC + hc) * H : (g * HC + hc + 1) * H],
                        start=(hc == 0),
                        stop=(hc == HC - 1),
                    )
                c = (bt * G + g) * H
                nc.scalar.copy(tsum_sb[:, c : c + H], ps[:])

        # Phase B: k-outer diag matmuls; pack 4 g-slices per (k, bt).
        for k in range(K):
            for bt in range(NBT):
                b0 = bt * P
                ot = opool.tile([P, G * H], out_dt)
                for g in range(G):
                    pd = psd.tile([P, H], F32)
                    for hc in range(HC):
                        xb = (k * HC + hc) * BS + b0
                        nc.tensor.matmul(
                            pd[:],
                            xt_sb[:, xb : xb + P],
                            dt_sb[:, (g * HC + hc) * H : (g * HC + hc + 1) * H],
                            start=(hc == 0),
                            stop=(hc == HC - 1),
                        )
                    c = (bt * G + g) * H
                    nc.vector.tensor_add(
                        ot[:, g * H : (g + 1) * H], pd[:], tsum_sb[:, c : c + H]
                    )
                dma_eng = nc.sync if (k * NBT + bt) % 2 == 0 else nc.scalar
                dma_eng.dma_start(y[bt, k, :, :], ot[:])

    nc.compile()
    return nc


def _get_nc():
    if "nc" not in _CACHE:
        _CACHE["nc"] = _build()
    return _CACHE["nc"]


def _prep_inputs(x, A, B):
    x = np.ascontiguousarray(np.asarray(x, dtype=np.float32))
    A = np.asarray(A, dtype=np.float32)
    B = np.asarray(B, dtype=np.float32)

    # [q, k, hc, b_global]
    xt_full = np.ascontiguousarray(
        x.T.reshape(K, HC, P, B_TOTAL).transpose(2, 0, 1, 3)
    )
    if MM_BF16:
        import ml_dtypes

        xt_full = xt_full.astype(ml_dtypes.bfloat16)
    s_full = x.reshape(B_TOTAL, K, H).sum(axis=1, dtype=np.float32)
    st_full = np.ascontiguousarray(
        s_full.T.reshape(HC, P, B_TOTAL).transpose(1, 0, 2)
    )
    # [q, g, hc, p]
    D = A - B
    dtw = np.ascontiguousarray(
        D.reshape(G, H, HC, P).transpose(3, 0, 2, 1)
    )
    btw = np.ascontiguousarray(
        B.reshape(G, H, HC, P).transpose(3, 0, 2, 1)
    )
    if MM_BF16:
        import ml_dtypes

        st_full = st_full.astype(ml_dtypes.bfloat16)
        dtw = dtw.astype(ml_dtypes.bfloat16)
        btw = btw.astype(ml_dtypes.bfloat16)

    in_maps = []
    for c in range(NCORES):
        cols = slice(c * BS, (c + 1) * BS)
        in_maps.append(
            {
                "xt": np.ascontiguousarray(xt_full[:, :, :, cols]),
                "st": np.ascontiguousarray(st_full[:, :, cols]),
                "dtw": dtw,
                "btw": btw,
            }
        )
    return in_maps


def _unpack_output(res):
    outs = []
    for c in range(NCORES):
        yd = np.asarray(res.results[c]["y"]).astype(np.float32)
        # [bt, k, p, g, pp] -> [bt, p, g, k, pp]
        yc = yd.reshape(NBT, K, P, G, H).transpose(0, 2, 3, 1, 4)
        outs.append(np.ascontiguousarray(yc).reshape(BS, G * K * H))
    return np.concatenate(outs, axis=0)


def _run(x, A, B, **run_kwargs):
    in_maps = _prep_inputs(x, A, B)
    nc = _get_nc()
    res = run_bass_kernel_spmd(nc, in_maps, list(range(NCORES)), **run_kwargs)
    return _unpack_output(res), res


def kernel(x, A, B):
    out, _ = _run(x, A, B)
    return out


# revision 19
# speedup vs baseline: 1.0697x; 1.0697x over previous
"""BlockEqLinear kernel for Trainium2 (8 NeuronCores, SPMD data-parallel over batch).

Math (reference):
    x: [4096, 4096] viewed as [B=4096, K=8, H=512]
    A, B: [G=4, H, H]
    out[b, g, k, :] = x_k[b] @ (A_g - B_g)^T + S[b] @ B_g^T,  S = sum_k x_k
    returned as [B, G*K*H] = [4096, 16384]

Strategy:
  - Shard batch across 8 cores (512 rows each); weights replicated.
  - Host-side layout prep: partition-major transposes so the contraction
    dim (h) is the partition dim on chip; precompute D = A - B.
  - Phase A (per core): tsum[bt,g] = S^T.T @ B_g^T  (16 groups of 4
    accumulating fp32r matmuls), evicted to SBUF.
  - Phase B: k-outer so x^T streams in just-in-time one k-slice at a
    time:  for k: for bt: for g: 4 matmuls -> psum; DVE adds tsum and
    packs 4 g-slices into one staging tile; one DMA per (k, bt).
  - fp32r (FP22) matmuls: full PE rate at moving-dim 512.
  - Output written as bf16 on device (halves write traffic; PE-bound
    after that), upcast to fp32 on host.
"""

import numpy as np

import concourse.bass as bass
import concourse.mybir as mybir
import concourse.tile as tile
from concourse import bacc
from concourse.bass_utils import run_bass_kernel_spmd
from contextlib import ExitStack

G, K, H = 4, 8, 512
B_TOTAL = 4096
NCORES = 8
BS = B_TOTAL // NCORES  # 512 batch rows per core
P = 128                 # partition dim
HC = H // P             # 4 contraction chunks per 512-dim h
NBT = BS // P           # 4 b-tiles per core

F32 = mybir.dt.float32
F32R = mybir.dt.float32r
BF16 = mybir.dt.bfloat16

OUT_BF16 = True         # write y as bf16 on device, upcast on host
MM_BF16 = True          # all matmul operands (x^T, S^T, weights) in bf16:
                        # halves input DMA, enables fast weight load
N_WARMUP = 5            # dummy PE matmuls to warm HAM during input DMA

_CACHE = {}


def _build():
    out_dt = BF16 if OUT_BF16 else F32

    nc = bacc.Bacc(
        "TRN2", target_bir_lowering=False, debug=False, num_devices=NCORES
    )

    # Host-packed, partition-major inputs:
    #   xt[q, k, hc, b]   = x[b, k*512 + hc*128 + q]
    #   st[q, hc, b]      = S[b, hc*128 + q]
    #   dtw[q, g, hc, p]  = (A - B)[g, p, hc*128 + q]
    #   btw[q, g, hc, p]  = B[g, p, hc*128 + q]
    x_dt = BF16 if MM_BF16 else F32R
    xt = nc.dram_tensor("xt", [P, K, HC, BS], x_dt, kind="ExternalInput")
    st = nc.dram_tensor("st", [P, HC, BS], x_dt, kind="ExternalInput")
    dtw = nc.dram_tensor("dtw", [P, G, HC, H], x_dt, kind="ExternalInput")
    btw = nc.dram_tensor("btw", [P, G, HC, H], x_dt, kind="ExternalInput")
    # y_dev[bt, k, p, g*512 + pp] = out[bt*128 + p, g, k, pp]
    y = nc.dram_tensor("y", [NBT, K, P, G * H], out_dt, kind="ExternalOutput")

    with tile.TileContext(nc) as tc, ExitStack() as ctx:
        wpool = ctx.enter_context(tc.tile_pool(name="w", bufs=1))
        xpool = ctx.enter_context(tc.tile_pool(name="x", bufs=1))
        tsump = ctx.enter_context(tc.tile_pool(name="tsum", bufs=1))
        opool = ctx.enter_context(tc.tile_pool(name="o", bufs=8))
        psd = ctx.enter_context(tc.tile_pool(name="psd", bufs=6, space="PSUM"))
        pss = ctx.enter_context(tc.tile_pool(name="pss", bufs=2, space="PSUM"))

        # PE warm-up scratch: zeroed tile for dummy matmuls (below) that
        # run while the first input DMAs are in flight, so HAM is at
        # K=8/8 (2.4 GHz) when the real matmul stream starts.
        scratch = wpool.tile([P, H], BF16)
        nc.gpsimd.memset(scratch[:], 0.0)

        # SBUF-resident tensors. Two HWDGE queues in parallel:
        # sync carries phase-A deps (st, btw), scalar carries phase-B
        # deps (dtw, xt) — so early arrivals overlap.
        st_sb = xpool.tile([P, HC * BS], x_dt)
        nc.sync.dma_start(st_sb[:], st[:, :, :])
        bt_sb = wpool.tile([P, G * HC * H], x_dt)
        for g in range(G):
            nc.sync.dma_start(
                bt_sb[:, g * HC * H : (g + 1) * HC * H], btw[:, g, :, :]
            )
        dt_sb = wpool.tile([P, G * HC * H], x_dt)
        for g in range(G):
            nc.sync.dma_start(
                dt_sb[:, g * HC * H : (g + 1) * HC * H], dtw[:, g, :, :]
            )
        xt_sb = xpool.tile([P, K * HC * BS], x_dt)
        for k in range(K):
            nc.sync.dma_start(
                xt_sb[:, k * HC * BS : (k + 1) * HC * BS], xt[:, k, :, :]
            )

        # Dummy warm-up matmuls (PE program order puts these before the
        # real stream; they execute during the input-DMA wait).
        warm_ps = pss.tile([P, H], F32, tag="ps")
        for i in range(N_WARMUP):
            nc.tensor.matmul(
                warm_ps[:],
                scratch[:, :P],
                scratch[:],
                start=True,
                stop=True,
            )

        # Phase A: tsum[bt, g] = S-tile @ B_g^T for all 16 (bt, g) pairs.
        # g-major so the first 16 matmuls need only st + btw[g=0].
        tsum_sb = tsump.tile([P, NBT * G * H], BF16)
        for g in range(G):
            for bt in range(NBT):
                b0 = bt * P
                ps = pss.tile([P, H], F32)
                for hc in range(HC):
                    nc.tensor.matmul(
                        ps[:],
                        st_sb[:, hc * BS + b0 : hc * BS + b0 + P],
                        bt_sb[:, (g * HC + hc) * H : (g * HC + hc + 1) * H],
                        start=(hc == 0),
                        stop=(hc == HC - 1),
                    )
                c = (bt * G + g) * H
                nc.scalar.copy(tsum_sb[:, c : c + H], ps[:])

        # Phase B: k-outer diag matmuls; pack 4 g-slices per (k, bt).
        for k in range(K):
            for bt in range(NBT):
                b0 = bt * P
                ot = opool.tile([P, G * H], out_dt)
                for g in range(G):
                    pd = psd.tile([P, H], F32)
                    for hc in range(HC):
                        xb = (k * HC + hc) * BS + b0
                        nc.tensor.matmul(
                            pd[:],
                            xt_sb[:, xb : xb + P],
                            dt_sb[:, (g * HC + hc) * H : (g * HC + hc + 1) * H],
                            start=(hc == 0),
                            stop=(hc == HC - 1),
                        )
                    c = (bt * G + g) * H
                    nc.vector.tensor_add(
                        ot[:, g * H : (g + 1) * H], pd[:], tsum_sb[:, c : c + H]
                    )
                nc.scalar.dma_start(y[bt, k, :, :], ot[:])

    nc.compile()
    return nc


def _get_nc():
    if "nc" not in _CACHE:
        _CACHE["nc"] = _build()
    return _CACHE["nc"]


def _prep_inputs(x, A, B):
    x = np.ascontiguousarray(np.asarray(x, dtype=np.float32))
    A = np.asarray(A, dtype=np.float32)
    B = np.asarray(B, dtype=np.float32)

    # [q, k, hc, b_global]
    xt_full = np.ascontiguousarray(
        x.T.reshape(K, HC, P, B_TOTAL).transpose(2, 0, 1, 3)
    )
    if MM_BF16:
        import ml_dtypes

        xt_full = xt_full.astype(ml_dtypes.bfloat16)
    s_full = x.reshape(B_TOTAL, K, H).sum(axis=1, dtype=np.float32)
    st_full = np.ascontiguousarray(
        s_full.T.reshape(HC, P, B_TOTAL).transpose(1, 0, 2)
    )
    # [q, g, hc, p]
    D = A - B
    dtw = np.ascontiguousarray(
        D.reshape(G, H, HC, P).transpose(3, 0, 2, 1)
    )
    btw = np.ascontiguousarray(
        B.reshape(G, H, HC, P).transpose(3, 0, 2, 1)
    )
    if MM_BF16:
        import ml_dtypes

        st_full = st_full.astype(ml_dtypes.bfloat16)
        dtw = dtw.astype(ml_dtypes.bfloat16)
        btw = btw.astype(ml_dtypes.bfloat16)

    in_maps = []
    for c in range(NCORES):
        cols = slice(c * BS, (c + 1) * BS)
        in_maps.append(
            {
                "xt": np.ascontiguousarray(xt_full[:, :, :, cols]),
                "st": np.ascontiguousarray(st_full[:, :, cols]),
                "dtw": dtw,
                "btw": btw,
            }
        )
    return in_maps


def _unpack_output(res):
    outs = []
    for c in range(NCORES):
        yd = np.asarray(res.results[c]["y"]).astype(np.float32)
        # [bt, k, p, g, pp] -> [bt, p, g, k, pp]
        yc = yd.reshape(NBT, K, P, G, H).transpose(0, 2, 3, 1, 4)
        outs.append(np.ascontiguousarray(yc).reshape(BS, G * K * H))
    return np.concatenate(outs, axis=0)


def _run(x, A, B, **run_kwargs):
    in_maps = _prep_inputs(x, A, B)
    nc = _get_nc()
    res = run_bass_kernel_spmd(nc, in_maps, list(range(NCORES)), **run_kwargs)
    return _unpack_output(res), res


def kernel(x, A, B):
    out, _ = _run(x, A, B)
    return out


# revision 20
# speedup vs baseline: 1.0758x; 1.0057x over previous
"""BlockEqLinear kernel for Trainium2 (8 NeuronCores, SPMD data-parallel over batch).

Math (reference):
    x: [4096, 4096] viewed as [B=4096, K=8, H=512]
    A, B: [G=4, H, H]
    out[b, g, k, :] = x_k[b] @ (A_g - B_g)^T + S[b] @ B_g^T,  S = sum_k x_k
    returned as [B, G*K*H] = [4096, 16384]

Strategy:
  - Shard batch across 8 cores (512 rows each); weights replicated.
  - Host-side layout prep: partition-major transposes so the contraction
    dim (h) is the partition dim on chip; precompute D = A - B.
  - Phase A (per core): tsum[bt,g] = S^T.T @ B_g^T  (16 groups of 4
    accumulating fp32r matmuls), evicted to SBUF.
  - Phase B: k-outer so x^T streams in just-in-time one k-slice at a
    time:  for k: for bt: for g: 4 matmuls -> psum; DVE adds tsum and
    packs 4 g-slices into one staging tile; one DMA per (k, bt).
  - fp32r (FP22) matmuls: full PE rate at moving-dim 512.
  - Output written as bf16 on device (halves write traffic; PE-bound
    after that), upcast to fp32 on host.
"""

import numpy as np

import concourse.bass as bass
import concourse.mybir as mybir
import concourse.tile as tile
from concourse import bacc
from concourse.bass_utils import run_bass_kernel_spmd
from contextlib import ExitStack

G, K, H = 4, 8, 512
B_TOTAL = 4096
NCORES = 8
BS = B_TOTAL // NCORES  # 512 batch rows per core
P = 128                 # partition dim
HC = H // P             # 4 contraction chunks per 512-dim h
NBT = BS // P           # 4 b-tiles per core

F32 = mybir.dt.float32
F32R = mybir.dt.float32r
BF16 = mybir.dt.bfloat16

OUT_BF16 = True         # write y as bf16 on device, upcast on host
MM_BF16 = True          # all matmul operands (x^T, S^T, weights) in bf16:
                        # halves input DMA, enables fast weight load
N_WARMUP = 6            # dummy PE matmuls to warm HAM during input DMA

_CACHE = {}


def _build():
    out_dt = BF16 if OUT_BF16 else F32

    nc = bacc.Bacc(
        "TRN2", target_bir_lowering=False, debug=False, num_devices=NCORES
    )

    # All inputs packed host-side into ONE partition-major tensor so a
    # few large column-chunk DMAs (one descriptor-gen each) feed SBUF:
    #   cols [0, 2048)            st[q, hc, b]     = S[b, hc*128+q]
    #   cols [2048, 10240)        btw[q, g, hc, p] = B[g, p, hc*128+q]
    #   cols [10240, 18432)       dtw[q, g, hc, p] = (A-B)[g, p, hc*128+q]
    #   cols [18432, 34816)       xt[q, k, hc, b]  = x[b, k*512+hc*128+q]
    x_dt = BF16 if MM_BF16 else F32R
    NCOL_ST = HC * BS
    NCOL_W = G * HC * H
    NCOL_X = K * HC * BS
    NCOL = NCOL_ST + 2 * NCOL_W + NCOL_X
    inp = nc.dram_tensor("inp", [P, NCOL], x_dt, kind="ExternalInput")
    # y_dev[bt, k, p, g*512 + pp] = out[bt*128 + p, g, k, pp]
    y = nc.dram_tensor("y", [NBT, K, P, G * H], out_dt, kind="ExternalOutput")

    with tile.TileContext(nc) as tc, ExitStack() as ctx:
        wpool = ctx.enter_context(tc.tile_pool(name="w", bufs=1))
        xpool = ctx.enter_context(tc.tile_pool(name="x", bufs=1))
        tsump = ctx.enter_context(tc.tile_pool(name="tsum", bufs=1))
        opool = ctx.enter_context(tc.tile_pool(name="o", bufs=8))
        psd = ctx.enter_context(tc.tile_pool(name="psd", bufs=6, space="PSUM"))
        pss = ctx.enter_context(tc.tile_pool(name="pss", bufs=2, space="PSUM"))

        # PE warm-up scratch: zeroed tile for dummy matmuls (below) that
        # run while the first input DMAs are in flight, so HAM is at
        # K=8/8 (2.4 GHz) when the real matmul stream starts.
        scratch = wpool.tile([P, H], BF16)
        nc.gpsimd.memset(scratch[:], 0.0)

        # One SBUF-resident input tile; chunked column DMAs in
        # consumption order (first chunk = st + btw[g0], exactly what
        # phase A's first group needs).
        in_sb = xpool.tile([P, NCOL], x_dt)
        O_ST = 0
        O_BT = NCOL_ST
        O_DT = NCOL_ST + NCOL_W
        O_XT = NCOL_ST + 2 * NCOL_W
        chunks = [
            (0, NCOL_ST + HC * H),                    # st + btw[g0]
            (NCOL_ST + HC * H, O_DT),                 # btw[g1..3]
            (O_DT, O_DT + NCOL_W // 2),               # dtw[g0..1]
            (O_DT + NCOL_W // 2, O_XT),               # dtw[g2..3]
        ]
        for k in range(K):
            chunks.append((O_XT + k * HC * BS, O_XT + (k + 1) * HC * BS))
        for c0, c1 in chunks:
            nc.sync.dma_start(in_sb[:, c0:c1], inp[:, c0:c1])
        st_sb = in_sb[:, O_ST : O_ST + NCOL_ST]
        bt_sb = in_sb[:, O_BT : O_BT + NCOL_W]
        dt_sb = in_sb[:, O_DT : O_DT + NCOL_W]
        xt_sb = in_sb[:, O_XT : O_XT + NCOL_X]

        # Dummy warm-up matmuls (PE program order puts these before the
        # real stream; they execute during the input-DMA wait).
        warm_ps = pss.tile([P, H], F32, tag="ps")
        for i in range(N_WARMUP):
            nc.tensor.matmul(
                warm_ps[:],
                scratch[:, :P],
                scratch[:],
                start=True,
                stop=True,
            )

        # Phase A: tsum[bt, g] = S-tile @ B_g^T for all 16 (bt, g) pairs.
        # g-major so the first 16 matmuls need only st + btw[g=0].
        tsum_sb = tsump.tile([P, NBT * G * H], BF16)
        for g in range(G):
            for bt in range(NBT):
                b0 = bt * P
                ps = pss.tile([P, H], F32)
                for hc in range(HC):
                    nc.tensor.matmul(
                        ps[:],
                        st_sb[:, hc * BS + b0 : hc * BS + b0 + P],
                        bt_sb[:, (g * HC + hc) * H : (g * HC + hc + 1) * H],
                        start=(hc == 0),
                        stop=(hc == HC - 1),
                    )
                c = (bt * G + g) * H
                nc.scalar.copy(tsum_sb[:, c : c + H], ps[:])

        # Phase B: k-outer diag matmuls; pack 4 g-slices per (k, bt).
        for k in range(K):
            for bt in range(NBT):
                b0 = bt * P
                ot = opool.tile([P, G * H], out_dt)
                for g in range(G):
                    pd = psd.tile([P, H], F32)
                    for hc in range(HC):
                        xb = (k * HC + hc) * BS + b0
                        nc.tensor.matmul(
                            pd[:],
                            xt_sb[:, xb : xb + P],
                            dt_sb[:, (g * HC + hc) * H : (g * HC + hc + 1) * H],
                            start=(hc == 0),
                            stop=(hc == HC - 1),
                        )
                    c = (bt * G + g) * H
                    nc.vector.tensor_add(
                        ot[:, g * H : (g + 1) * H], pd[:], tsum_sb[:, c : c + H]
                    )
                nc.scalar.dma_start(y[bt, k, :, :], ot[:])

    nc.compile()
    return nc


def _get_nc():
    if "nc" not in _CACHE:
        _CACHE["nc"] = _build()
    return _CACHE["nc"]


def _prep_inputs(x, A, B):
    x = np.ascontiguousarray(np.asarray(x, dtype=np.float32))
    A = np.asarray(A, dtype=np.float32)
    B = np.asarray(B, dtype=np.float32)

    # [q, k, hc, b_global]
    xt_full = np.ascontiguousarray(
        x.T.reshape(K, HC, P, B_TOTAL).transpose(2, 0, 1, 3)
    )
    if MM_BF16:
        import ml_dtypes

        xt_full = xt_full.astype(ml_dtypes.bfloat16)
    s_full = x.reshape(B_TOTAL, K, H).sum(axis=1, dtype=np.float32)
    st_full = np.ascontiguousarray(
        s_full.T.reshape(HC, P, B_TOTAL).transpose(1, 0, 2)
    )
    # [q, g, hc, p]
    D = A - B
    dtw = np.ascontiguousarray(
        D.reshape(G, H, HC, P).transpose(3, 0, 2, 1)
    )
    btw = np.ascontiguousarray(
        B.reshape(G, H, HC, P).transpose(3, 0, 2, 1)
    )
    if MM_BF16:
        import ml_dtypes

        st_full = st_full.astype(ml_dtypes.bfloat16)
        dtw = dtw.astype(ml_dtypes.bfloat16)
        btw = btw.astype(ml_dtypes.bfloat16)

    wflat = np.concatenate(
        [btw.reshape(P, G * HC * H), dtw.reshape(P, G * HC * H)], axis=1
    )
    in_maps = []
    for c in range(NCORES):
        cols = slice(c * BS, (c + 1) * BS)
        packed = np.concatenate(
            [
                st_full[:, :, cols].reshape(P, HC * BS),
                wflat,
                xt_full[:, :, :, cols].reshape(P, K * HC * BS),
            ],
            axis=1,
        )
        in_maps.append({"inp": np.ascontiguousarray(packed)})
    return in_maps


def _unpack_output(res):
    outs = []
    for c in range(NCORES):
        yd = np.asarray(res.results[c]["y"]).astype(np.float32)
        # [bt, k, p, g, pp] -> [bt, p, g, k, pp]
        yc = yd.reshape(NBT, K, P, G, H).transpose(0, 2, 3, 1, 4)
        outs.append(np.ascontiguousarray(yc).reshape(BS, G * K * H))
    return np.concatenate(outs, axis=0)


def _run(x, A, B, **run_kwargs):
    in_maps = _prep_inputs(x, A, B)
    nc = _get_nc()
    res = run_bass_kernel_spmd(nc, in_maps, list(range(NCORES)), **run_kwargs)
    return _unpack_output(res), res


def kernel(x, A, B):
    out, _ = _run(x, A, B)
    return out
